# revision 3
# baseline (speedup 1.0000x reference)
"""Trainium2 Bass kernel v4.1 for nn_Co_Attention (B=256, Nu=Ni=512, D=64).

Math:  S_b = u_fea[b] @ K2 @ i_fea[b].T,  K2 = Wu.T @ M @ Wi  (biases zero)
       p_u = softmax(S.max(axis=2), axis=1);  p_i = softmax(S.max(axis=1), axis=1)

Design (constraints discovered by microbenchmarking this hardware):
 - tensor_tensor_reduce crashes at runtime (even the v1 SPLIT_N path);
   ops can read at most ONE PSUM operand; Pool(gpsimd) supports no
   elementwise/reduce ops (only memset / partition_* customs / DMA issue);
   ACT cannot max; DVE reduce = 1 elem/cycle/partition @0.96GHz; compute
   engines cannot address partition bases other than 0/32/64/96.
 - Single S-pass in fp16 (split hi/lo operands: ~6.9e-3 max-rel, gate 2e-2).
   All matmuls use K=64 stationaries placed in both partition halves so
   consecutive instructions run on PE row groups h0/h64 concurrently.
 - ACT evacuates the 4 S tiles PSUM->SBUF (two [128,1024] copies).
 - DVE: ONE strided rowmax over [128,(4),512] -> 4 USC cols per batch,
   then a 2-op tensor_tensor max tree -> T[128,512].
 - Pool: partition_all_reduce(max) over T -> i_score row; DMA copies row 0
   into IV[b] (SBUF->SBUF).
 - Tail: u via PE transpose of USC; i softmaxes IV [32,512] directly.
"""

import os
import numpy as np

B, NU, NI, D = 256, 512, 512, 64
NCORES = 8
BPC = B // NCORES  # 32

_BUILD_CACHE = {}
last_run_info = {}


def _build_kernel_v4(bpc):
    import concourse.bass as bass
    import concourse.tile as tile
    from concourse import bacc, mybir, bass_isa

    f32 = mybir.dt.float32
    f16 = mybir.dt.float16
    X = mybir.AxisListType.X
    MAX = mybir.AluOpType.max
    Exp = mybir.ActivationFunctionType.Exp

    nc = bacc.Bacc("TRN2", debug=False, enable_asserts=True,
                   target_bir_lowering=False)

    up2_d = nc.dram_tensor("up2", [bpc, 128, 512], f16, kind="ExternalInput")
    ih2_d = nc.dram_tensor("ih2", [bpc, 128, 1024], f16, kind="ExternalInput")
    k2h_d = nc.dram_tensor("k2h", [128, 64], f16, kind="ExternalInput")
    k2l_d = nc.dram_tensor("k2l", [128, 64], f16, kind="ExternalInput")
    ident_d = nc.dram_tensor("ident", [128, 128], f32, kind="ExternalInput")
    pu_d = nc.dram_tensor("pu", [bpc, 512], f32, kind="ExternalOutput")
    pi_d = nc.dram_tensor("pi", [bpc, 512], f32, kind="ExternalOutput")

    scw = 4 * bpc  # 128

    with tile.TileContext(nc) as tc:
        with (
            tc.tile_pool(name="consts", bufs=1) as cpool,
            tc.tile_pool(name="inp", bufs=8) as ipool,
            tc.tile_pool(name="gsb", bufs=2) as gpool,
            tc.tile_pool(name="scopy", bufs=3) as scpool_sb,
            tc.tile_pool(name="tt", bufs=4) as ttpool,
            tc.tile_pool(name="pout", bufs=3) as popool,
            tc.tile_pool(name="score", bufs=1) as scpool,
            tc.tile_pool(name="pg", bufs=2, space="PSUM") as pgpool,
            tc.tile_pool(name="pss", bufs=3, space="PSUM") as psspool,
            tc.tile_pool(name="tail", bufs=2) as tailpool,
        ):
            # k2h/k2l: K2.T stacked in both partition halves [128, 64]
            k2h = cpool.tile([128, 64], f16, tag="k2h")
            k2l = cpool.tile([128, 64], f16, tag="k2l")
            ident = cpool.tile([128, 128], f32, tag="ident")
            nc.sync.dma_start(k2h[:], k2h_d.ap())
            nc.sync.dma_start(k2l[:], k2l_d.ap())
            nc.sync.dma_start(ident[:], ident_d.ap())

            USC = scpool.tile([128, scw], f32, tag="usc")
            IV = scpool.tile([bpc, 512], f32, tag="iv")

            def load_inputs(b):
                up2 = ipool.tile([128, 512], f16, tag="up2")
                ih2 = ipool.tile([128, 1024], f16, tag="ih2")
                nc.sync.dma_start(up2[:], up2_d.ap()[b])
                nc.sync.dma_start(ih2[:], ih2_d.ap()[b])
                return up2, ih2

            def proj(ih2):
                # G = K2 @ iT, duplicated in both halves; K=64 stationaries at
                # partition bases 0 / 64 -> h0/h64 run concurrently.
                ihd, ild = ih2[:, 0:512], ih2[:, 512:1024]
                g_ps = pgpool.tile([128, 512], f32, tag="g")
                terms = ((k2h, ihd, True, False), (k2h, ild, False, False),
                         (k2l, ihd, False, True))
                for w, x, st, sp in terms:
                    nc.tensor.matmul(g_ps[0:64, :], w[0:64, :], x[0:64, :],
                                     start=st, stop=sp)
                    nc.tensor.matmul(g_ps[64:128, :], w[64:128, :],
                                     x[64:128, :], start=st, stop=sp)
                g2x = gpool.tile([128, 512], f16, tag="g2x")
                nc.scalar.copy(g2x[:], g_ps[:])
                return g2x

            # prologue: inputs + projection for batch 0 (and input for 1)
            ins = [load_inputs(0)]
            g2xs = [proj(ins[0][1])]

            for b in range(bpc):
                up2, _ = ins[b]
                g2x = g2xs[b]
                uph, upl = up2[:, 0:256], up2[:, 256:512]

                # ---- S-pass: tiles t: pair0=(t0,t1) h0, pair1=(t2,t3) h64.
                pair0 = psspool.tile([128, 1024], f32, tag="s")
                pair1 = psspool.tile([128, 1024], f32, tag="s")
                for up, st, sp in ((uph, True, False), (upl, False, True)):
                    for q in (0, 1):
                        nc.tensor.matmul(pair0[:, 512 * q:512 * q + 512],
                                         up[0:64, 128 * q:128 * q + 128],
                                         g2x[0:64, :], start=st, stop=sp)
                        nc.tensor.matmul(pair1[:, 512 * q:512 * q + 512],
                                         up[64:128, 128 * q:128 * q + 128],
                                         g2x[64:128, :], start=st, stop=sp)

                # lookahead: inputs + projection for batch b+1 queue BEFORE
                # this batch's big ACT evacs so the next S-pass isn't gated.
                if b + 1 < bpc:
                    ins.append(load_inputs(b + 1))
                    g2xs.append(proj(ins[b + 1][1]))

                # ---- ACT evacuates all 4 tiles into one [128,2048] copy
                # sc tile order: t0 | t1 | t2 | t3 (nu chunks 0..3 x 128)
                sc = scpool_sb.tile([128, 2048], f32, tag="sc")
                nc.scalar.copy(sc[:, 0:1024], pair0[:])
                nc.scalar.copy(sc[:, 1024:2048], pair1[:])

                # ---- u_score: ONE strided rowmax over [128,(4),512]
                nc.vector.reduce_max(
                    USC[:, b:b + 3 * bpc + 1:bpc],
                    sc[:].rearrange("p (t n) -> p t n", t=4), axis=X)

                # ---- T = elementwise max over the 4 tiles (2 ops)
                tt = ttpool.tile([128, 1024], f32, tag="tt")
                nc.vector.tensor_tensor(tt[:], sc[:, 0:1024],
                                        sc[:, 1024:2048], op=MAX)
                tf = ttpool.tile([128, 512], f32, tag="tf")
                nc.vector.tensor_tensor(tf[:], tt[:, 0:512],
                                        tt[:, 512:1024], op=MAX)

                # ---- i_score row: partition max on gpsimd; row 0 -> IV[b]
                po = popool.tile([128, 512], f32, tag="po")
                nc.gpsimd.partition_all_reduce(
                    po[:], tf[:], channels=128,
                    reduce_op=bass_isa.ReduceOp.max)
                nc.gpsimd.dma_start(IV[b:b + 1, :], po[0:1, :])

            # ---- softmax tails ----
            # u: transpose USC -> [scw,128] -> v [bpc,512] -> softmax
            sct_ps = pgpool.tile([scw, 128], f32, tag="g")
            nc.tensor.transpose(sct_ps[:], USC[:], ident[:])
            sct = tailpool.tile([scw, 128], f32, tag="sct")
            nc.scalar.copy(sct[:], sct_ps[:])
            v = tailpool.tile([bpc, 512], f32, tag="v")
            for t in range(4):
                nc.sync.dma_start(v[:, 128 * t:128 * (t + 1)],
                                  sct[bpc * t:bpc * (t + 1), :])

            for src, out_d in ((v, pu_d), (IV, pi_d)):
                m = tailpool.tile([bpc, 1], f32, tag="m")
                nc.vector.reduce_max(m[:], src[:], axis=X)
                negm = tailpool.tile([bpc, 1], f32, tag="negm")
                nc.scalar.mul(negm[:], m[:], -1.0)
                e = tailpool.tile([bpc, 512], f32, tag="e")
                esum = tailpool.tile([bpc, 1], f32, tag="esum")
                nc.scalar.activation(e[:], src[:], Exp, bias=negm[:], scale=1.0,
                                     accum_out=esum[:])
                rs = tailpool.tile([bpc, 1], f32, tag="rs")
                nc.vector.reciprocal(rs[:], esum[:])
                pout = tailpool.tile([bpc, 512], f32, tag="p")
                nc.vector.tensor_scalar_mul(pout[:], e[:], rs[:])
                nc.sync.dma_start(out_d.ap(), pout[:])

    nc.compile()
    return nc


def _get_kernel(bpc):
    if bpc not in _BUILD_CACHE:
        _BUILD_CACHE[bpc] = _build_kernel_v4(bpc)
    return _BUILD_CACHE[bpc]


def _host_pack(xT):  # [n, 64, 512] -> packed [n, 128, 256]
    n = xT.shape[0]
    return np.ascontiguousarray(
        xT.reshape(n, 64, 2, 256).transpose(0, 2, 1, 3).reshape(n, 128, 256))


def _split16(x):
    h = x.astype(np.float16)
    l = (x.astype(np.float32) - h.astype(np.float32)).astype(np.float16)
    return h, l


def kernel(u_fea, i_fea, M, Wu, bu, Wi, bi):
    u_fea = np.asarray(u_fea, dtype=np.float32)
    i_fea = np.asarray(i_fea, dtype=np.float32)
    M = np.asarray(M, dtype=np.float32)
    Wu = np.asarray(Wu, dtype=np.float32)
    Wi = np.asarray(Wi, dtype=np.float32)
    bu = np.asarray(bu, dtype=np.float32)
    bi = np.asarray(bi, dtype=np.float32)
    assert not np.any(bu) and not np.any(bi), "nonzero biases unsupported"

    from concourse.bass_utils import run_bass_kernel_spmd

    K2 = (Wu.T.astype(np.float64) @ M.astype(np.float64)
          @ Wi.astype(np.float64)).astype(np.float32)
    K2h, K2l = _split16(K2)
    k2h = np.ascontiguousarray(np.concatenate([K2h.T, K2h.T], axis=0))  # [128,64]
    k2l = np.ascontiguousarray(np.concatenate([K2l.T, K2l.T], axis=0))
    ident = np.eye(128, dtype=np.float32)

    uT = np.ascontiguousarray(u_fea.transpose(0, 2, 1))   # [B, 64, 512]
    iT = np.ascontiguousarray(i_fea.transpose(0, 2, 1))
    up = _host_pack(uT)                                   # [B, 128, 256] f32
    uph, upl = _split16(up)
    up2 = np.ascontiguousarray(np.concatenate([uph, upl], axis=2))  # [B,128,512]
    iTh, iTl = _split16(iT)                               # [B, 64, 512] f16
    ihd = np.concatenate([iTh, iTh], axis=1)              # [B, 128, 512]
    ild = np.concatenate([iTl, iTl], axis=1)
    ih2 = np.ascontiguousarray(np.concatenate([ihd, ild], axis=2))  # [B,128,1024]

    nc = _get_kernel(BPC)

    in_maps = []
    for c in range(NCORES):
        sl = slice(c * BPC, (c + 1) * BPC)
        in_maps.append({
            "up2": up2[sl], "ih2": ih2[sl],
            "k2h": k2h, "k2l": k2l,
            "ident": ident,
        })

    trace = os.environ.get("CO_ATTN_TRACE", "0") == "1"
    res = run_bass_kernel_spmd(nc, in_maps, core_ids=list(range(NCORES)),
                               trace=trace)
    last_run_info.clear()
    last_run_info.update({
        "exec_time_ns": res.exec_time_ns,
        "mean_exec_time_ns": res.mean_exec_time_ns,
        "results_obj": res,
    })

    p_u = np.concatenate([res.results[c]["pu"] for c in range(NCORES)], axis=0)
    p_i = np.concatenate([res.results[c]["pi"] for c in range(NCORES)], axis=0)
    return p_u[:, :, None].astype(np.float32), p_i[:, :, None].astype(np.float32)


# revision 5
# speedup vs baseline: 1.0045x; 1.0045x over previous
"""Trainium2 Bass kernel v4.1 for nn_Co_Attention (B=256, Nu=Ni=512, D=64).

Math:  S_b = u_fea[b] @ K2 @ i_fea[b].T,  K2 = Wu.T @ M @ Wi  (biases zero)
       p_u = softmax(S.max(axis=2), axis=1);  p_i = softmax(S.max(axis=1), axis=1)

Design (constraints discovered by microbenchmarking this hardware):
 - tensor_tensor_reduce crashes at runtime (even the v1 SPLIT_N path);
   ops can read at most ONE PSUM operand; Pool(gpsimd) supports no
   elementwise/reduce ops (only memset / partition_* customs / DMA issue);
   ACT cannot max; DVE reduce = 1 elem/cycle/partition @0.96GHz; compute
   engines cannot address partition bases other than 0/32/64/96.
 - Single S-pass in fp16 (split hi/lo operands: ~6.9e-3 max-rel, gate 2e-2).
   All matmuls use K=64 stationaries placed in both partition halves so
   consecutive instructions run on PE row groups h0/h64 concurrently.
 - ACT evacuates the 4 S tiles PSUM->SBUF (two [128,1024] copies).
 - DVE: ONE strided rowmax over [128,(4),512] -> 4 USC cols per batch,
   then a 2-op tensor_tensor max tree -> T[128,512].
 - Pool: partition_all_reduce(max) over T -> i_score row; DMA copies row 0
   into IV[b] (SBUF->SBUF).
 - Tail: u via PE transpose of USC; i softmaxes IV [32,512] directly.
"""

import os
import numpy as np

B, NU, NI, D = 256, 512, 512, 64
NCORES = 8
BPC = B // NCORES  # 32

_BUILD_CACHE = {}
last_run_info = {}


def _build_kernel_v4(bpc):
    import concourse.bass as bass
    import concourse.tile as tile
    from concourse import bacc, mybir, bass_isa

    f32 = mybir.dt.float32
    f16 = mybir.dt.float16
    X = mybir.AxisListType.X
    MAX = mybir.AluOpType.max
    Exp = mybir.ActivationFunctionType.Exp

    nc = bacc.Bacc("TRN2", debug=False, enable_asserts=True,
                   target_bir_lowering=False)

    up2_d = nc.dram_tensor("up2", [bpc, 128, 512], f16, kind="ExternalInput")
    ih2_d = nc.dram_tensor("ih2", [bpc, 128, 1024], f16, kind="ExternalInput")
    k2h_d = nc.dram_tensor("k2h", [128, 64], f16, kind="ExternalInput")
    k2l_d = nc.dram_tensor("k2l", [128, 64], f16, kind="ExternalInput")
    ident_d = nc.dram_tensor("ident", [128, 128], f32, kind="ExternalInput")
    pu_d = nc.dram_tensor("pu", [bpc, 512], f32, kind="ExternalOutput")
    pi_d = nc.dram_tensor("pi", [bpc, 512], f32, kind="ExternalOutput")

    scw = 4 * bpc  # 128

    with tile.TileContext(nc) as tc:
        with (
            tc.tile_pool(name="consts", bufs=1) as cpool,
            tc.tile_pool(name="inp", bufs=8) as ipool,
            tc.tile_pool(name="gsb", bufs=2) as gpool,
            tc.tile_pool(name="scopy", bufs=2) as scpool_sb,
            tc.tile_pool(name="tt", bufs=2) as ttpool,
            tc.tile_pool(name="pout", bufs=2) as popool,
            tc.tile_pool(name="score", bufs=1) as scpool,
            tc.tile_pool(name="pg", bufs=2, space="PSUM") as pgpool,
            tc.tile_pool(name="pss", bufs=3, space="PSUM") as psspool,
            tc.tile_pool(name="tail", bufs=2) as tailpool,
        ):
            # k2h/k2l: K2.T stacked in both partition halves [128, 64]
            k2h = cpool.tile([128, 64], f16, tag="k2h")
            k2l = cpool.tile([128, 64], f16, tag="k2l")
            ident = cpool.tile([128, 128], f32, tag="ident")
            nc.sync.dma_start(k2h[:], k2h_d.ap())
            nc.sync.dma_start(k2l[:], k2l_d.ap())
            nc.sync.dma_start(ident[:], ident_d.ap())

            USC = scpool.tile([128, scw], f32, tag="usc")
            IV = scpool.tile([bpc, 512], f32, tag="iv")

            def load_inputs(b):
                up2 = ipool.tile([128, 512], f16, tag="up2")
                ih2 = ipool.tile([128, 1024], f16, tag="ih2")
                nc.sync.dma_start(up2[:], up2_d.ap()[b])
                nc.sync.dma_start(ih2[:], ih2_d.ap()[b])
                return up2, ih2

            def proj(ih2):
                # G = K2 @ iT, duplicated in both halves; K=64 stationaries at
                # partition bases 0 / 64 -> h0/h64 run concurrently.
                ihd, ild = ih2[:, 0:512], ih2[:, 512:1024]
                g_ps = pgpool.tile([128, 512], f32, tag="g")
                terms = ((k2h, ihd, True, False), (k2h, ild, False, False),
                         (k2l, ihd, False, True))
                for w, x, st, sp in terms:
                    nc.tensor.matmul(g_ps[0:64, :], w[0:64, :], x[0:64, :],
                                     start=st, stop=sp)
                    nc.tensor.matmul(g_ps[64:128, :], w[64:128, :],
                                     x[64:128, :], start=st, stop=sp)
                g2x = gpool.tile([128, 512], f16, tag="g2x")
                nc.scalar.copy(g2x[:], g_ps[:])
                return g2x

            # prologue: inputs + projection for batch 0 (and input for 1)
            ins = [load_inputs(0)]
            g2xs = [proj(ins[0][1])]

            for b in range(bpc):
                up2, _ = ins[b]
                g2x = g2xs[b]
                uph, upl = up2[:, 0:256], up2[:, 256:512]

                # ---- S-pass: tiles t: pair0=(t0,t1) h0, pair1=(t2,t3) h64.
                pair0 = psspool.tile([128, 1024], f32, tag="s")
                pair1 = psspool.tile([128, 1024], f32, tag="s")
                for up, st, sp in ((uph, True, False), (upl, False, True)):
                    for q in (0, 1):
                        nc.tensor.matmul(pair0[:, 512 * q:512 * q + 512],
                                         up[0:64, 128 * q:128 * q + 128],
                                         g2x[0:64, :], start=st, stop=sp)
                        nc.tensor.matmul(pair1[:, 512 * q:512 * q + 512],
                                         up[64:128, 128 * q:128 * q + 128],
                                         g2x[64:128, :], start=st, stop=sp)

                # lookahead: inputs + projection for batch b+1 queue BEFORE
                # this batch's big ACT evacs so the next S-pass isn't gated.
                if b + 1 < bpc:
                    ins.append(load_inputs(b + 1))
                    g2xs.append(proj(ins[b + 1][1]))

                # ---- ACT evacuates all 4 tiles into one [128,2048] copy
                # sc tile order: t0 | t1 | t2 | t3 (nu chunks 0..3 x 128)
                # rowmax is split per evac half so the DVE starts as soon as
                # the first ACT copy lands instead of waiting for both.
                sc = scpool_sb.tile([128, 2048], f32, tag="sc")
                nc.scalar.copy(sc[:, 0:1024], pair0[:])
                nc.vector.reduce_max(
                    USC[:, b:b + bpc + 1:bpc],
                    sc[:, 0:1024].rearrange("p (t n) -> p t n", t=2), axis=X)
                nc.scalar.copy(sc[:, 1024:2048], pair1[:])
                nc.vector.reduce_max(
                    USC[:, b + 2 * bpc:b + 3 * bpc + 1:bpc],
                    sc[:, 1024:2048].rearrange("p (t n) -> p t n", t=2), axis=X)

                # ---- T = elementwise max over the 4 tiles (2 ops)
                tt = ttpool.tile([128, 1024], f32, tag="tt")
                nc.vector.tensor_tensor(tt[:], sc[:, 0:1024],
                                        sc[:, 1024:2048], op=MAX)
                tf = ttpool.tile([128, 512], f32, tag="tf")
                nc.vector.tensor_tensor(tf[:], tt[:, 0:512],
                                        tt[:, 512:1024], op=MAX)

                # ---- i_score row: partition max on gpsimd; row 0 -> IV[b]
                po = popool.tile([128, 512], f32, tag="po")
                nc.gpsimd.partition_all_reduce(
                    po[:], tf[:], channels=128,
                    reduce_op=bass_isa.ReduceOp.max)
                nc.gpsimd.dma_start(IV[b:b + 1, :], po[0:1, :])

            # ---- softmax tails ----
            # u: transpose USC -> [scw,128] -> v [bpc,512] -> softmax
            sct_ps = pgpool.tile([scw, 128], f32, tag="g")
            nc.tensor.transpose(sct_ps[:], USC[:], ident[:])
            sct = tailpool.tile([scw, 128], f32, tag="sct")
            nc.scalar.copy(sct[:], sct_ps[:])
            v = tailpool.tile([bpc, 512], f32, tag="v")
            for t in range(4):
                nc.sync.dma_start(v[:, 128 * t:128 * (t + 1)],
                                  sct[bpc * t:bpc * (t + 1), :])

            for src, out_d in ((v, pu_d), (IV, pi_d)):
                m = tailpool.tile([bpc, 1], f32, tag="m")
                nc.vector.reduce_max(m[:], src[:], axis=X)
                negm = tailpool.tile([bpc, 1], f32, tag="negm")
                nc.scalar.mul(negm[:], m[:], -1.0)
                e = tailpool.tile([bpc, 512], f32, tag="e")
                esum = tailpool.tile([bpc, 1], f32, tag="esum")
                nc.scalar.activation(e[:], src[:], Exp, bias=negm[:], scale=1.0,
                                     accum_out=esum[:])
                rs = tailpool.tile([bpc, 1], f32, tag="rs")
                nc.vector.reciprocal(rs[:], esum[:])
                pout = tailpool.tile([bpc, 512], f32, tag="p")
                nc.vector.tensor_scalar_mul(pout[:], e[:], rs[:])
                nc.sync.dma_start(out_d.ap(), pout[:])

    nc.compile()
    return nc


def _get_kernel(bpc):
    if bpc not in _BUILD_CACHE:
        _BUILD_CACHE[bpc] = _build_kernel_v4(bpc)
    return _BUILD_CACHE[bpc]


def _host_pack(xT):  # [n, 64, 512] -> packed [n, 128, 256]
    n = xT.shape[0]
    return np.ascontiguousarray(
        xT.reshape(n, 64, 2, 256).transpose(0, 2, 1, 3).reshape(n, 128, 256))


def _split16(x):
    h = x.astype(np.float16)
    l = (x.astype(np.float32) - h.astype(np.float32)).astype(np.float16)
    return h, l


def kernel(u_fea, i_fea, M, Wu, bu, Wi, bi):
    u_fea = np.asarray(u_fea, dtype=np.float32)
    i_fea = np.asarray(i_fea, dtype=np.float32)
    M = np.asarray(M, dtype=np.float32)
    Wu = np.asarray(Wu, dtype=np.float32)
    Wi = np.asarray(Wi, dtype=np.float32)
    bu = np.asarray(bu, dtype=np.float32)
    bi = np.asarray(bi, dtype=np.float32)
    assert not np.any(bu) and not np.any(bi), "nonzero biases unsupported"

    from concourse.bass_utils import run_bass_kernel_spmd

    K2 = (Wu.T.astype(np.float64) @ M.astype(np.float64)
          @ Wi.astype(np.float64)).astype(np.float32)
    K2h, K2l = _split16(K2)
    k2h = np.ascontiguousarray(np.concatenate([K2h.T, K2h.T], axis=0))  # [128,64]
    k2l = np.ascontiguousarray(np.concatenate([K2l.T, K2l.T], axis=0))
    ident = np.eye(128, dtype=np.float32)

    uT = np.ascontiguousarray(u_fea.transpose(0, 2, 1))   # [B, 64, 512]
    iT = np.ascontiguousarray(i_fea.transpose(0, 2, 1))
    up = _host_pack(uT)                                   # [B, 128, 256] f32
    uph, upl = _split16(up)
    up2 = np.ascontiguousarray(np.concatenate([uph, upl], axis=2))  # [B,128,512]
    iTh, iTl = _split16(iT)                               # [B, 64, 512] f16
    ihd = np.concatenate([iTh, iTh], axis=1)              # [B, 128, 512]
    ild = np.concatenate([iTl, iTl], axis=1)
    ih2 = np.ascontiguousarray(np.concatenate([ihd, ild], axis=2))  # [B,128,1024]

    nc = _get_kernel(BPC)

    in_maps = []
    for c in range(NCORES):
        sl = slice(c * BPC, (c + 1) * BPC)
        in_maps.append({
            "up2": up2[sl], "ih2": ih2[sl],
            "k2h": k2h, "k2l": k2l,
            "ident": ident,
        })

    trace = os.environ.get("CO_ATTN_TRACE", "0") == "1"
    res = run_bass_kernel_spmd(nc, in_maps, core_ids=list(range(NCORES)),
                               trace=trace)
    last_run_info.clear()
    last_run_info.update({
        "exec_time_ns": res.exec_time_ns,
        "mean_exec_time_ns": res.mean_exec_time_ns,
        "results_obj": res,
    })

    p_u = np.concatenate([res.results[c]["pu"] for c in range(NCORES)], axis=0)
    p_i = np.concatenate([res.results[c]["pi"] for c in range(NCORES)], axis=0)
    return p_u[:, :, None].astype(np.float32), p_i[:, :, None].astype(np.float32)
